# revision 5
# baseline (speedup 1.0000x reference)
"""Expert-parallel MoE FFN kernel for Trainium2 (Bass/Tile).

Problem: y[b,e,n,:] = gelu(x[b,e,n,:] @ w1[e] + b1[e]) @ w2[e] + b2[e]
Shapes:  x (2,8,2048,1024), w1 (8,1024,4096), b1 (8,4096),
         w2 (8,4096,1024), b2 (8,1024)  -> out (2,8,2048,1024) fp32.

Sharding: expert-parallel, one expert per NeuronCore (8 cores).  Each core
processes its expert's 4096 tokens through the full FFN locally; no
cross-core communication.

Per-core dataflow (all matmuls in float32r at N=512 -> full PE rate):
  Phase 1:  xT = transpose(x_e)  (PE transpose, 128x128 tiles)
            hT[h,t] = gelu(sum_d w1[d,h] * xT[d,t] + b1[h])   (hT: [H,T])
            hT staged to DRAM (doesn't fit SBUF alongside weights).
  Phase 2:  y[t,d] = sum_h hT[h,t] * w2[h,d] (+ b2)
The hT (activation-transposed) layout means the big [H,T] intermediate is
produced and consumed with no transposes; only x needs a transpose on the
way in, and y comes out in natural [T,D] layout.

Scheduling notes:
 - GEMM1 runs group-outer (token-chunk groups) so the first-half xT tile can
   be freed at the GEMM1 midpoint; the leading slice of w2 loads into that
   space, hiding most of the phase-2 weight-load latency.
 - PSUM: GEMM1 uses [128, 2*512] psum tiles double-buffered (4 banks) so the
   GELU drain of one group overlaps the next group's matmuls; transposes use
   2 more banks; phase 2 reuses freed banks for [128, 1024] x2.
"""

import numpy as np
from contextlib import ExitStack

import concourse.bass as bass
import concourse.mybir as mybir
import concourse.tile as tile
from concourse import bacc
from concourse.bass_utils import run_bass_kernel_spmd
from concourse.masks import make_identity

P = 128
F32 = mybir.dt.float32
F32R = mybir.dt.float32r

# Full-size problem constants (hardcoded; the grading harness calls
# kernel(**inputs) with exactly these shapes).
B, E, N, D, H = 2, 8, 2048, 1024, 4096
N_CORES = 8


def emit_expert_ffn(tc, x, w1, b1, w2, b2, y, hT_st, T, D_, H_, TCH=512, DCH=512,
                    G=2, use_b2=False):
    """Emit one expert's FFN. x:[T,D] w1:[D,H] b1:[H] w2:[H,D] b2:[D] y:[T,D].

    hT_st: [H, T] DRAM staging tile (float32r) for the transposed activation.
    TCH: token chunk (matmul moving free dim) for GEMM1.
    DCH: output-dim chunk for GEMM2 (one PSUM bank).
    G:   token chunks per PSUM accumulation group in GEMM1.
    """
    nc = tc.nc
    NT = T // P          # token subtiles
    ND = D_ // P         # contraction tiles for GEMM1
    NH = H_ // P         # h tiles
    NCH = T // TCH       # token chunks
    NG = NCH // G        # chunk groups
    NDC = D_ // DCH      # output chunks for GEMM2
    NG_A = NG // 2       # groups reading the first-half xT tile
    W2A = min(8, NH)     # leading w2 h-tiles loaded early

    assert T % P == 0 and D_ % P == 0 and H_ % P == 0
    assert T % TCH == 0 and NCH % G == 0 and D_ % DCH == 0 and NG % 2 == 0
    T_half = NG_A * G * TCH

    hT_r = hT_st.rearrange("(ht p) t -> p ht t", p=P)
    w2_r = w2.rearrange("(ht p) d -> p ht d", p=P)

    with ExitStack() as es:
        const_pool = es.enter_context(tc.tile_pool(name="const", bufs=1, side="right"))
        ident = const_pool.tile([P, P], F32)
        make_identity(nc, ident)
        b1_sb = const_pool.tile([P, NH], F32)
        nc.sync.dma_start(b1_sb[:], b1.rearrange("(ht p) -> p ht", p=P))
        if use_b2:
            b2_sb = const_pool.tile([P, D_], F32)
            nc.sync.dma_start(b2_sb[:], b2.unsqueeze(0).broadcast_to([P, D_]))

        # -------- Phase 1: xT transpose + hT = gelu(w1.T @ xT + b1) --------
        p1 = ExitStack()
        xraw_pool = p1.enter_context(tc.tile_pool(name="xraw_pool", bufs=3, side="left"))
        w1_pool = p1.enter_context(tc.tile_pool(name="w1_pool", bufs=6, side="left"))
        hout_pool = p1.enter_context(tc.tile_pool(name="hout_pool", bufs=3, side="left"))
        ptp_pool = p1.enter_context(tc.tile_pool(name="ptp_pool", bufs=2,
                                                 space="PSUM", side="left"))
        ph_pool = p1.enter_context(tc.tile_pool(name="ph_pool", bufs=2,
                                                space="PSUM", side="left"))
        xTb_pool = p1.enter_context(tc.tile_pool(name="xTb_pool", bufs=1, side="left"))
        xTa_es = ExitStack()
        xTa_pool = xTa_es.enter_context(tc.tile_pool(name="xTa_pool", bufs=1, side="left"))

        xT_a = xTa_pool.tile([P, ND, T_half], F32R)
        xT_b = xTb_pool.tile([P, ND, T - T_half], F32R)

        def xT_slice(dt, t0, t1):
            if t1 <= T_half:
                return xT_a[:, dt, t0:t1]
            assert t0 >= T_half
            return xT_b[:, dt, t0 - T_half:t1 - T_half]

        with nc.named_scope("transpose"):
            for tsub in range(NT):
                x_raw = xraw_pool.tile([P, D_], F32, name="x_raw")
                nc.sync.dma_start(x_raw[:], x[tsub * P:(tsub + 1) * P, :])
                for dt in range(ND):
                    ptp = ptp_pool.tile([P, P], F32, name="ptp")
                    nc.tensor.transpose(ptp[:], x_raw[:, dt * P:(dt + 1) * P],
                                        ident[:])
                    nc.vector.tensor_copy(
                        xT_slice(dt, tsub * P, (tsub + 1) * P), ptp[:])

        def gemm1_group(g):
            psum_h = ph_pool.tile([P, G * TCH], F32, name="psum_h")
            for ht in range(NH):
                for dt in range(ND):
                    w1_t = w1_pool.tile([P, P], F32R, name="w1_t")
                    nc.sync.dma_start(
                        w1_t[:],
                        w1[dt * P:(dt + 1) * P, ht * P:(ht + 1) * P])
                    for i in range(G):
                        tc0 = (g * G + i) * TCH
                        nc.tensor.matmul(
                            psum_h[:, i * TCH:(i + 1) * TCH],
                            w1_t[:],
                            xT_slice(dt, tc0, tc0 + TCH),
                            start=(dt == 0), stop=(dt == ND - 1))
                hT_out = hout_pool.tile([P, G * TCH], F32R, name="hT_out")
                for i in range(G):
                    nc.scalar.activation(
                        hT_out[:, i * TCH:(i + 1) * TCH],
                        psum_h[:, i * TCH:(i + 1) * TCH],
                        mybir.ActivationFunctionType.Gelu_apprx_tanh,
                        bias=b1_sb[:, ht:ht + 1], scale=1.0)
                nc.sync.dma_start(
                    hT_st[ht * P:(ht + 1) * P,
                          g * G * TCH:(g + 1) * G * TCH],
                    hT_out[:])
                if ht != NH - 1:
                    psum_h = ph_pool.tile([P, G * TCH], F32, name="psum_h")

        with nc.named_scope("gemm1a"):
            for g in range(NG_A):
                gemm1_group(g)
        # first-half xT is dead; free its SBUF for the leading w2 slice.
        xTa_es.close()

        p2 = ExitStack()
        w2a_pool = p2.enter_context(tc.tile_pool(name="w2a_pool", bufs=1, side="right"))
        w2a = w2a_pool.tile([P, W2A, D_], F32R)
        for ht in range(W2A):
            nc.sync.dma_start(w2a[:, ht, :], w2_r[:, ht, :])

        with nc.named_scope("gemm1b"):
            for g in range(NG_A, NG):
                gemm1_group(g)
        p1.close()

        # -------- Phase 2: y = hT.T @ w2 (+ b2) ----------------------------
        w2b_pool = p2.enter_context(tc.tile_pool(name="w2b_pool", bufs=1, side="right"))
        hTin_pool = p2.enter_context(tc.tile_pool(name="hTin_pool", bufs=2, side="right"))
        out_pool = p2.enter_context(tc.tile_pool(name="out_pool", bufs=3, side="right"))
        po_pool = p2.enter_context(tc.tile_pool(name="po_pool", bufs=2,
                                                space="PSUM", side="right"))
        if NH > W2A:
            w2b = w2b_pool.tile([P, NH - W2A, D_], F32R)
            for ht in range(W2A, NH):
                nc.sync.dma_start(w2b[:, ht - W2A, :], w2_r[:, ht, :])

        def w2_sb(ht):
            return w2a[:, ht, :] if ht < W2A else w2b[:, ht - W2A, :]

        with nc.named_scope("gemm2"):
            for tt in range(NT):
                hT_in = hTin_pool.tile([P, NH, P], F32R, name="hT_in")
                nc.sync.dma_start(hT_in[:], hT_r[:, :, tt * P:(tt + 1) * P])
                psum_o = po_pool.tile([P, D_], F32, name="psum_o")
                for ht in range(NH):
                    for dc in range(NDC):
                        nc.tensor.matmul(
                            psum_o[:, dc * DCH:(dc + 1) * DCH],
                            hT_in[:, ht, :],
                            w2_sb(ht)[:, dc * DCH:(dc + 1) * DCH],
                            start=(ht == 0), stop=(ht == NH - 1))
                out_sb = out_pool.tile([P, D_], F32, name="out_sb")
                if use_b2:
                    nc.vector.tensor_add(out_sb[:], psum_o[:], b2_sb[:])
                else:
                    nc.scalar.copy(out_sb[:], psum_o[:])
                nc.sync.dma_start(y[tt * P:(tt + 1) * P, :], out_sb[:])
        p2.close()


def build_module(T, D_, H_, TCH=512, DCH=512, use_b2=False):
    nc = bacc.Bacc(None, target_bir_lowering=False)
    x = nc.dram_tensor("x", [T, D_], F32, kind="ExternalInput")
    w1 = nc.dram_tensor("w1", [D_, H_], F32R, kind="ExternalInput")
    b1 = nc.dram_tensor("b1", [H_], F32, kind="ExternalInput")
    w2 = nc.dram_tensor("w2", [H_, D_], F32R, kind="ExternalInput")
    if use_b2:
        b2 = nc.dram_tensor("b2", [D_], F32, kind="ExternalInput")
    else:
        b2 = None
    y = nc.dram_tensor("y", [T, D_], F32, kind="ExternalOutput")

    with tile.TileContext(nc) as tc:
        with tc.tile_pool(name="dram_st", bufs=1, space="DRAM") as dram_pool:
            hT_st = dram_pool.tile([H_, T], F32R)
            emit_expert_ffn(tc, x[:], w1[:], b1[:], w2[:],
                            b2[:] if use_b2 else None, y[:], hT_st,
                            T, D_, H_, TCH=TCH, DCH=DCH, use_b2=use_b2)
    nc.compile()
    return nc


_module_cache = {}


def _get_module(key):
    if key not in _module_cache:
        T, D_, H_, use_b2 = key
        _module_cache[key] = build_module(T, D_, H_, use_b2=use_b2)
    return _module_cache[key]


def run_moe(x, w1, b1, w2, b2, trace=False):
    """x:(B,E,N,D) w1:(E,D,H) b1:(E,H) w2:(E,H,D) b2:(E,D) -> (B,E,N,D)."""
    Bx, Ex, Nx, Dx = x.shape
    Hx = w1.shape[2]
    T = Bx * Nx
    use_b2 = bool(np.any(b2))
    nc = _get_module((T, Dx, Hx, use_b2))

    in_maps = []
    for e in range(Ex):
        m = {
            "x": np.ascontiguousarray(x[:, e]).reshape(T, Dx),
            "w1": np.ascontiguousarray(w1[e]),
            "b1": np.ascontiguousarray(b1[e]),
            "w2": np.ascontiguousarray(w2[e]),
        }
        if use_b2:
            m["b2"] = np.ascontiguousarray(b2[e])
        in_maps.append(m)

    br = run_bass_kernel_spmd(nc, in_maps, core_ids=list(range(Ex)),
                              trace=trace)
    ys = np.stack([br.results[e]["y"] for e in range(Ex)], axis=0)  # [E,T,D]
    out = ys.reshape(Ex, Bx, Nx, Dx).reshape(Bx, Ex, Nx, Dx)
    return (out, br) if trace else (out, None)


def kernel(x, w1, b1, w2, b2):
    out, _ = run_moe(np.asarray(x), np.asarray(w1), np.asarray(b1),
                     np.asarray(w2), np.asarray(b2))
    return out


# revision 6
# speedup vs baseline: 1.3816x; 1.3816x over previous
"""Expert-parallel MoE FFN kernel for Trainium2 (Bass/Tile).

Problem: y[b,e,n,:] = gelu(x[b,e,n,:] @ w1[e] + b1[e]) @ w2[e] + b2[e]
Shapes:  x (2,8,2048,1024), w1 (8,1024,4096), b1 (8,4096),
         w2 (8,4096,1024), b2 (8,1024)  -> out (2,8,2048,1024) fp32.

Sharding: expert-parallel, one expert per NeuronCore (8 cores).  Each core
processes its expert's 4096 tokens through the full FFN locally; no
cross-core communication.

Per-core dataflow (all matmuls in float32r at N=512 -> full PE rate):
  Phase 1:  xT = transpose(x_e)  (PE transpose, 128x128 tiles)
            hT[h,t] = gelu(sum_d w1[d,h] * xT[d,t] + b1[h])   (hT: [H,T])
            hT staged to DRAM (doesn't fit SBUF alongside weights).
  Phase 2:  y[t,d] = sum_h hT[h,t] * w2[h,d] (+ b2)
The hT (activation-transposed) layout means the big [H,T] intermediate is
produced and consumed with no transposes; only x needs a transpose on the
way in, and y comes out in natural [T,D] layout.

Scheduling notes:
 - GEMM1 runs group-outer (token-chunk groups) so the first-half xT tile can
   be freed at the GEMM1 midpoint; the leading slice of w2 loads into that
   space, hiding most of the phase-2 weight-load latency.
 - PSUM: GEMM1 uses [128, 2*512] psum tiles double-buffered (4 banks) so the
   GELU drain of one group overlaps the next group's matmuls; transposes use
   2 more banks; phase 2 reuses freed banks for [128, 1024] x2.
"""

import numpy as np
from contextlib import ExitStack

import concourse.bass as bass
import concourse.mybir as mybir
import concourse.tile as tile
from concourse import bacc
from concourse.bass_utils import run_bass_kernel_spmd
from concourse.masks import make_identity

P = 128
F32 = mybir.dt.float32
F32R = mybir.dt.float32r

# Full-size problem constants (hardcoded; the grading harness calls
# kernel(**inputs) with exactly these shapes).
B, E, N, D, H = 2, 8, 2048, 1024, 4096
N_CORES = 8


def emit_expert_ffn(tc, x, w1, b1, w2, b2, y, hT_st, T, D_, H_, TCH=512, DCH=512,
                    G=4, use_b2=False):
    """Emit one expert's FFN. x:[T,D] w1:[D,H] b1:[H] w2:[H,D] b2:[D] y:[T,D].

    hT_st: [H, T] DRAM staging tile (float32r) for the transposed activation.
    TCH: token chunk (matmul moving free dim) for GEMM1.
    DCH: output-dim chunk for GEMM2 (one PSUM bank).
    G:   token chunks per PSUM accumulation group in GEMM1.
    """
    nc = tc.nc
    NT = T // P          # token subtiles
    ND = D_ // P         # contraction tiles for GEMM1
    NH = H_ // P         # h tiles
    NCH = T // TCH       # token chunks
    NG = NCH // G        # chunk groups
    NDC = D_ // DCH      # output chunks for GEMM2
    NG_A = NG // 2       # groups reading the first-half xT tile
    W2A = min(16, NH)    # leading w2 h-tiles loaded during gemm1b

    assert T % P == 0 and D_ % P == 0 and H_ % P == 0
    assert T % TCH == 0 and NCH % G == 0 and D_ % DCH == 0 and NG % 2 == 0
    T_half = NG_A * G * TCH

    hT_r = hT_st.rearrange("(ht p) t -> p ht t", p=P)
    w2_r = w2.rearrange("(ht p) d -> p ht d", p=P)

    with ExitStack() as es:
        const_pool = es.enter_context(tc.tile_pool(name="const", bufs=1, side="right"))
        ident = const_pool.tile([P, P], F32)
        make_identity(nc, ident)
        b1_sb = const_pool.tile([P, NH], F32)
        nc.sync.dma_start(b1_sb[:], b1.rearrange("(ht p) -> p ht", p=P))
        if use_b2:
            b2_sb = const_pool.tile([P, D_], F32)
            nc.sync.dma_start(b2_sb[:], b2.unsqueeze(0).broadcast_to([P, D_]))

        # -------- Phase 1: xT transpose + hT = gelu(w1.T @ xT + b1) --------
        p1 = ExitStack()
        w1_pool = p1.enter_context(tc.tile_pool(name="w1_pool", bufs=2, side="left"))
        hout_pool = p1.enter_context(tc.tile_pool(name="hout_pool", bufs=2, side="left"))
        xTb_pool = p1.enter_context(tc.tile_pool(name="xTb_pool", bufs=1, side="left"))
        xTa_es = ExitStack()
        xTa_pool = xTa_es.enter_context(tc.tile_pool(name="xTa_pool", bufs=1, side="left"))

        xT_a = xTa_pool.tile([P, ND, T_half], F32R)
        xT_b = xTb_pool.tile([P, ND, T - T_half], F32R)

        def xT_slice(dt, t0, t1):
            if t1 <= T_half:
                return xT_a[:, dt, t0:t1]
            assert t0 >= T_half
            return xT_b[:, dt, t0 - T_half:t1 - T_half]

        with nc.named_scope("transpose"):
            with (
                tc.tile_pool(name="xraw_pool", bufs=3, side="left") as xraw_pool,
                tc.tile_pool(name="ptp_pool", bufs=2, space="PSUM",
                             side="left") as ptp_pool,
            ):
                for tsub in range(NT):
                    x_raw = xraw_pool.tile([P, D_], F32, name="x_raw")
                    nc.sync.dma_start(x_raw[:], x[tsub * P:(tsub + 1) * P, :])
                    for dt in range(ND):
                        ptp = ptp_pool.tile([P, P], F32, name="ptp")
                        nc.tensor.transpose(ptp[:],
                                            x_raw[:, dt * P:(dt + 1) * P],
                                            ident[:])
                        nc.vector.tensor_copy(
                            xT_slice(dt, tsub * P, (tsub + 1) * P), ptp[:])

        # GEMM1 psum: [128, G*512] = 4 banks, double buffered = all 8 banks
        # (transpose psum pool closed above).
        ph_pool = p1.enter_context(tc.tile_pool(name="ph_pool", bufs=2,
                                                space="PSUM", side="left"))

        HTG = 4              # h-tiles per w1 chunk load (2KB DMA rows)
        assert NH % HTG == 0

        def gemm1_group(g):
            for htg in range(NH // HTG):
                w1_t = w1_pool.tile([P, ND, HTG * P], F32R, name="w1_t")
                for dt in range(ND):
                    nc.sync.dma_start(
                        w1_t[:, dt, :],
                        w1[dt * P:(dt + 1) * P,
                           htg * HTG * P:(htg + 1) * HTG * P])
                for hl in range(HTG):
                    ht = htg * HTG + hl
                    psum_h = ph_pool.tile([P, G * TCH], F32, name="psum_h")
                    for dt in range(ND):
                        for i in range(G):
                            tc0 = (g * G + i) * TCH
                            nc.tensor.matmul(
                                psum_h[:, i * TCH:(i + 1) * TCH],
                                w1_t[:, dt, hl * P:(hl + 1) * P],
                                xT_slice(dt, tc0, tc0 + TCH),
                                start=(dt == 0), stop=(dt == ND - 1))
                    hT_out = hout_pool.tile([P, G * TCH], F32R, name="hT_out")
                    for i in range(G):
                        nc.scalar.activation(
                            hT_out[:, i * TCH:(i + 1) * TCH],
                            psum_h[:, i * TCH:(i + 1) * TCH],
                            mybir.ActivationFunctionType.Gelu_apprx_tanh,
                            bias=b1_sb[:, ht:ht + 1], scale=1.0)
                    nc.sync.dma_start(
                        hT_st[ht * P:(ht + 1) * P,
                              g * G * TCH:(g + 1) * G * TCH],
                        hT_out[:])

        with nc.named_scope("gemm1a"):
            for g in range(NG_A):
                gemm1_group(g)
        # first-half xT is dead; free its SBUF for the leading w2 slice.
        xTa_es.close()

        p2 = ExitStack()
        w2a_pool = p2.enter_context(tc.tile_pool(name="w2a_pool", bufs=1, side="right"))
        w2a = w2a_pool.tile([P, W2A, D_], F32R)
        for ht in range(W2A):
            nc.sync.dma_start(w2a[:, ht, :], w2_r[:, ht, :])

        with nc.named_scope("gemm1b"):
            for g in range(NG_A, NG):
                gemm1_group(g)
        p1.close()

        # -------- Phase 2: y = hT.T @ w2 (+ b2) ----------------------------
        w2b_pool = p2.enter_context(tc.tile_pool(name="w2b_pool", bufs=1, side="right"))
        hTin_pool = p2.enter_context(tc.tile_pool(name="hTin_pool", bufs=2, side="right"))
        out_pool = p2.enter_context(tc.tile_pool(name="out_pool", bufs=3, side="right"))
        po_pool = p2.enter_context(tc.tile_pool(name="po_pool", bufs=2,
                                                space="PSUM", side="right"))
        if NH > W2A:
            w2b = w2b_pool.tile([P, NH - W2A, D_], F32R)
            for ht in range(W2A, NH):
                nc.sync.dma_start(w2b[:, ht - W2A, :], w2_r[:, ht, :])

        def w2_sb(ht):
            return w2a[:, ht, :] if ht < W2A else w2b[:, ht - W2A, :]

        with nc.named_scope("gemm2"):
            for tt in range(NT):
                hT_in = hTin_pool.tile([P, NH, P], F32R, name="hT_in")
                nc.sync.dma_start(hT_in[:], hT_r[:, :, tt * P:(tt + 1) * P])
                psum_o = po_pool.tile([P, D_], F32, name="psum_o")
                for ht in range(NH):
                    for dc in range(NDC):
                        nc.tensor.matmul(
                            psum_o[:, dc * DCH:(dc + 1) * DCH],
                            hT_in[:, ht, :],
                            w2_sb(ht)[:, dc * DCH:(dc + 1) * DCH],
                            start=(ht == 0), stop=(ht == NH - 1))
                out_sb = out_pool.tile([P, D_], F32, name="out_sb")
                if use_b2:
                    nc.vector.tensor_add(out_sb[:], psum_o[:], b2_sb[:])
                else:
                    nc.scalar.copy(out_sb[:], psum_o[:])
                nc.sync.dma_start(y[tt * P:(tt + 1) * P, :], out_sb[:])
        p2.close()


def build_module(T, D_, H_, TCH=512, DCH=512, use_b2=False):
    nc = bacc.Bacc(None, target_bir_lowering=False)
    x = nc.dram_tensor("x", [T, D_], F32, kind="ExternalInput")
    w1 = nc.dram_tensor("w1", [D_, H_], F32R, kind="ExternalInput")
    b1 = nc.dram_tensor("b1", [H_], F32, kind="ExternalInput")
    w2 = nc.dram_tensor("w2", [H_, D_], F32R, kind="ExternalInput")
    if use_b2:
        b2 = nc.dram_tensor("b2", [D_], F32, kind="ExternalInput")
    else:
        b2 = None
    y = nc.dram_tensor("y", [T, D_], F32, kind="ExternalOutput")

    with tile.TileContext(nc) as tc:
        with tc.tile_pool(name="dram_st", bufs=1, space="DRAM") as dram_pool:
            hT_st = dram_pool.tile([H_, T], F32R)
            emit_expert_ffn(tc, x[:], w1[:], b1[:], w2[:],
                            b2[:] if use_b2 else None, y[:], hT_st,
                            T, D_, H_, TCH=TCH, DCH=DCH, use_b2=use_b2)
    nc.compile()
    return nc


_module_cache = {}


def _get_module(key):
    if key not in _module_cache:
        T, D_, H_, use_b2 = key
        _module_cache[key] = build_module(T, D_, H_, use_b2=use_b2)
    return _module_cache[key]


def run_moe(x, w1, b1, w2, b2, trace=False):
    """x:(B,E,N,D) w1:(E,D,H) b1:(E,H) w2:(E,H,D) b2:(E,D) -> (B,E,N,D)."""
    Bx, Ex, Nx, Dx = x.shape
    Hx = w1.shape[2]
    T = Bx * Nx
    use_b2 = bool(np.any(b2))
    nc = _get_module((T, Dx, Hx, use_b2))

    in_maps = []
    for e in range(Ex):
        m = {
            "x": np.ascontiguousarray(x[:, e]).reshape(T, Dx),
            "w1": np.ascontiguousarray(w1[e]),
            "b1": np.ascontiguousarray(b1[e]),
            "w2": np.ascontiguousarray(w2[e]),
        }
        if use_b2:
            m["b2"] = np.ascontiguousarray(b2[e])
        in_maps.append(m)

    br = run_bass_kernel_spmd(nc, in_maps, core_ids=list(range(Ex)),
                              trace=trace)
    ys = np.stack([br.results[e]["y"] for e in range(Ex)], axis=0)  # [E,T,D]
    out = ys.reshape(Ex, Bx, Nx, Dx).reshape(Bx, Ex, Nx, Dx)
    return (out, br) if trace else (out, None)


def kernel(x, w1, b1, w2, b2):
    out, _ = run_moe(np.asarray(x), np.asarray(w1), np.asarray(b1),
                     np.asarray(w2), np.asarray(b2))
    return out


# revision 7
# speedup vs baseline: 1.4437x; 1.0450x over previous
"""Expert-parallel MoE FFN kernel for Trainium2 (Bass/Tile).

Problem: y[b,e,n,:] = gelu(x[b,e,n,:] @ w1[e] + b1[e]) @ w2[e] + b2[e]
Shapes:  x (2,8,2048,1024), w1 (8,1024,4096), b1 (8,4096),
         w2 (8,4096,1024), b2 (8,1024)  -> out (2,8,2048,1024) fp32.

Sharding: expert-parallel, one expert per NeuronCore (8 cores).  Each core
processes its expert's 4096 tokens through the full FFN locally; no
cross-core communication.

Per-core dataflow (all matmuls in float32r at N=512 -> full PE rate):
  Phase 1:  xT = transpose(x_e)  (PE transpose, 128x128 tiles)
            hT[h,t] = gelu(sum_d w1[d,h] * xT[d,t] + b1[h])   (hT: [H,T])
            hT staged to DRAM (doesn't fit SBUF alongside weights).
  Phase 2:  y[t,d] = sum_h hT[h,t] * w2[h,d] (+ b2)
The hT (activation-transposed) layout means the big [H,T] intermediate is
produced and consumed with no transposes; only x needs a transpose on the
way in, and y comes out in natural [T,D] layout.

Scheduling notes:
 - GEMM1 runs group-outer (token-chunk groups) so the first-half xT tile can
   be freed at the GEMM1 midpoint; the leading slice of w2 loads into that
   space, hiding most of the phase-2 weight-load latency.
 - PSUM: GEMM1 uses [128, 2*512] psum tiles double-buffered (4 banks) so the
   GELU drain of one group overlaps the next group's matmuls; transposes use
   2 more banks; phase 2 reuses freed banks for [128, 1024] x2.
"""

import numpy as np
from contextlib import ExitStack

import concourse.bass as bass
import concourse.mybir as mybir
import concourse.tile as tile
from concourse import bacc
from concourse.bass_utils import run_bass_kernel_spmd
from concourse.masks import make_identity

P = 128
F32 = mybir.dt.float32
F32R = mybir.dt.float32r

# Full-size problem constants (hardcoded; the grading harness calls
# kernel(**inputs) with exactly these shapes).
B, E, N, D, H = 2, 8, 2048, 1024, 4096
N_CORES = 8


def emit_expert_ffn(tc, x, w1, b1, w2, b2, y, hT_st, T, D_, H_, TCH=512, DCH=512,
                    G=4, use_b2=False):
    """Emit one expert's FFN. x:[T,D] w1:[D,H] b1:[H] w2:[H,D] b2:[D] y:[T,D].

    hT_st: [H, T] DRAM staging tile (float32r) for the transposed activation.
    TCH: token chunk (matmul moving free dim) for GEMM1.
    DCH: output-dim chunk for GEMM2 (one PSUM bank).
    G:   token chunks per PSUM accumulation group in GEMM1.
    """
    nc = tc.nc
    NT = T // P          # token subtiles
    ND = D_ // P         # contraction tiles for GEMM1
    NH = H_ // P         # h tiles
    NCH = T // TCH       # token chunks
    NG = NCH // G        # chunk groups
    NDC = D_ // DCH      # output chunks for GEMM2
    NG_A = NG // 2       # groups reading the first-half xT tile
    W2A = min(16, NH)    # leading w2 h-tiles loaded during gemm1b

    assert T % P == 0 and D_ % P == 0 and H_ % P == 0
    assert T % TCH == 0 and NCH % G == 0 and D_ % DCH == 0 and NG % 2 == 0
    T_half = NG_A * G * TCH

    hT_r = hT_st.rearrange("(ht p) t -> p ht t", p=P)
    w2_r = w2.rearrange("(ht p) d -> p ht d", p=P)

    with ExitStack() as es:
        const_pool = es.enter_context(tc.tile_pool(name="const", bufs=1, side="right"))
        ident = const_pool.tile([P, P], F32)
        make_identity(nc, ident)
        b1_sb = const_pool.tile([P, NH], F32)
        nc.sync.dma_start(b1_sb[:], b1.rearrange("(ht p) -> p ht", p=P))
        if use_b2:
            b2_sb = const_pool.tile([P, D_], F32)
            nc.sync.dma_start(b2_sb[:], b2.unsqueeze(0).broadcast_to([P, D_]))

        # -------- Phase 1: xT transpose + hT = gelu(w1.T @ xT + b1) --------
        p1 = ExitStack()
        w1_pool = p1.enter_context(tc.tile_pool(name="w1_pool", bufs=2, side="left"))
        hout_pool = p1.enter_context(tc.tile_pool(name="hout_pool", bufs=2, side="left"))
        xTb_pool = p1.enter_context(tc.tile_pool(name="xTb_pool", bufs=1, side="left"))
        xTa_es = ExitStack()
        xTa_pool = xTa_es.enter_context(tc.tile_pool(name="xTa_pool", bufs=1, side="left"))

        xT_a = xTa_pool.tile([P, ND, T_half], F32R)
        xT_b = xTb_pool.tile([P, ND, T - T_half], F32R)

        def xT_slice(dt, t0, t1):
            if t1 <= T_half:
                return xT_a[:, dt, t0:t1]
            assert t0 >= T_half
            return xT_b[:, dt, t0 - T_half:t1 - T_half]

        with nc.named_scope("transpose"):
            with (
                tc.tile_pool(name="xraw_pool", bufs=3, side="left") as xraw_pool,
                tc.tile_pool(name="ptp_pool", bufs=4, space="PSUM",
                             side="left") as ptp_pool,
            ):
                for tsub in range(NT):
                    x_raw = xraw_pool.tile([P, D_], F32, name="x_raw")
                    nc.sync.dma_start(x_raw[:], x[tsub * P:(tsub + 1) * P, :])
                    for dt in range(ND):
                        ptp = ptp_pool.tile([P, P], F32, name="ptp")
                        nc.tensor.transpose(ptp[:],
                                            x_raw[:, dt * P:(dt + 1) * P],
                                            ident[:])
                        nc.vector.tensor_copy(
                            xT_slice(dt, tsub * P, (tsub + 1) * P), ptp[:])

        # GEMM1 psum: [128, G*512] = 4 banks, double buffered = all 8 banks
        # (transpose psum pool closed above).
        ph_pool = p1.enter_context(tc.tile_pool(name="ph_pool", bufs=2,
                                                space="PSUM", side="left"))

        HTG = 4              # h-tiles per w1 chunk load (2KB DMA rows)
        assert NH % HTG == 0

        def gemm1_group(g):
            for htg in range(NH // HTG):
                w1_t = w1_pool.tile([P, ND, HTG * P], F32R, name="w1_t")
                for dt in range(ND):
                    nc.sync.dma_start(
                        w1_t[:, dt, :],
                        w1[dt * P:(dt + 1) * P,
                           htg * HTG * P:(htg + 1) * HTG * P])
                for hl in range(HTG):
                    ht = htg * HTG + hl
                    psum_h = ph_pool.tile([P, G * TCH], F32, name="psum_h")
                    for dt in range(ND):
                        for i in range(G):
                            tc0 = (g * G + i) * TCH
                            nc.tensor.matmul(
                                psum_h[:, i * TCH:(i + 1) * TCH],
                                w1_t[:, dt, hl * P:(hl + 1) * P],
                                xT_slice(dt, tc0, tc0 + TCH),
                                start=(dt == 0), stop=(dt == ND - 1))
                    hT_out = hout_pool.tile([P, G * TCH], F32R, name="hT_out")
                    for i in range(G):
                        nc.scalar.activation(
                            hT_out[:, i * TCH:(i + 1) * TCH],
                            psum_h[:, i * TCH:(i + 1) * TCH],
                            mybir.ActivationFunctionType.Gelu_apprx_tanh,
                            bias=b1_sb[:, ht:ht + 1], scale=1.0)
                    nc.sync.dma_start(
                        hT_st[ht * P:(ht + 1) * P,
                              g * G * TCH:(g + 1) * G * TCH],
                        hT_out[:])

        with nc.named_scope("gemm1a"):
            for g in range(NG_A):
                gemm1_group(g)
        # first-half xT is dead; free its SBUF for the leading w2 slice.
        xTa_es.close()

        p2 = ExitStack()
        w2a_pool = p2.enter_context(tc.tile_pool(name="w2a_pool", bufs=1, side="right"))
        w2a = w2a_pool.tile([P, W2A, D_], F32R)
        for ht in range(W2A):
            nc.sync.dma_start(w2a[:, ht, :], w2_r[:, ht, :])

        with nc.named_scope("gemm1b"):
            for g in range(NG_A, NG):
                gemm1_group(g)
        p1.close()

        # -------- Phase 2: y = hT.T @ w2 (+ b2) ----------------------------
        w2b_pool = p2.enter_context(tc.tile_pool(name="w2b_pool", bufs=1, side="right"))
        hTin_pool = p2.enter_context(tc.tile_pool(name="hTin_pool", bufs=2, side="right"))
        out_pool = p2.enter_context(tc.tile_pool(name="out_pool", bufs=3, side="right"))
        po_pool = p2.enter_context(tc.tile_pool(name="po_pool", bufs=2,
                                                space="PSUM", side="right"))
        if NH > W2A:
            w2b = w2b_pool.tile([P, NH - W2A, D_], F32R)
            for ht in range(W2A, NH):
                nc.sync.dma_start(w2b[:, ht - W2A, :], w2_r[:, ht, :])

        def w2_sb(ht):
            return w2a[:, ht, :] if ht < W2A else w2b[:, ht - W2A, :]

        with nc.named_scope("gemm2"):
            for tt in range(NT):
                hT_in = hTin_pool.tile([P, NH, P], F32R, name="hT_in")
                for hq in range(0, NH, NH // 4):
                    nc.sync.dma_start(
                        hT_in[:, hq:hq + NH // 4, :],
                        hT_r[:, hq:hq + NH // 4, tt * P:(tt + 1) * P])
                psum_o = po_pool.tile([P, D_], F32, name="psum_o")
                for ht in range(NH):
                    for dc in range(NDC):
                        nc.tensor.matmul(
                            psum_o[:, dc * DCH:(dc + 1) * DCH],
                            hT_in[:, ht, :],
                            w2_sb(ht)[:, dc * DCH:(dc + 1) * DCH],
                            start=(ht == 0), stop=(ht == NH - 1))
                out_sb = out_pool.tile([P, D_], F32, name="out_sb")
                if use_b2:
                    nc.vector.tensor_add(out_sb[:], psum_o[:], b2_sb[:])
                else:
                    nc.scalar.copy(out_sb[:], psum_o[:])
                nc.sync.dma_start(y[tt * P:(tt + 1) * P, :], out_sb[:])
        p2.close()


def build_module(T, D_, H_, TCH=512, DCH=512, use_b2=False):
    nc = bacc.Bacc(None, target_bir_lowering=False)
    x = nc.dram_tensor("x", [T, D_], F32, kind="ExternalInput")
    w1 = nc.dram_tensor("w1", [D_, H_], F32R, kind="ExternalInput")
    b1 = nc.dram_tensor("b1", [H_], F32, kind="ExternalInput")
    w2 = nc.dram_tensor("w2", [H_, D_], F32R, kind="ExternalInput")
    if use_b2:
        b2 = nc.dram_tensor("b2", [D_], F32, kind="ExternalInput")
    else:
        b2 = None
    y = nc.dram_tensor("y", [T, D_], F32, kind="ExternalOutput")

    with tile.TileContext(nc) as tc:
        with tc.tile_pool(name="dram_st", bufs=1, space="DRAM") as dram_pool:
            hT_st = dram_pool.tile([H_, T], F32R)
            emit_expert_ffn(tc, x[:], w1[:], b1[:], w2[:],
                            b2[:] if use_b2 else None, y[:], hT_st,
                            T, D_, H_, TCH=TCH, DCH=DCH, use_b2=use_b2)
    nc.compile()
    return nc


_module_cache = {}


def _get_module(key):
    if key not in _module_cache:
        T, D_, H_, use_b2 = key
        _module_cache[key] = build_module(T, D_, H_, use_b2=use_b2)
    return _module_cache[key]


def run_moe(x, w1, b1, w2, b2, trace=False):
    """x:(B,E,N,D) w1:(E,D,H) b1:(E,H) w2:(E,H,D) b2:(E,D) -> (B,E,N,D)."""
    Bx, Ex, Nx, Dx = x.shape
    Hx = w1.shape[2]
    T = Bx * Nx
    use_b2 = bool(np.any(b2))
    nc = _get_module((T, Dx, Hx, use_b2))

    in_maps = []
    for e in range(Ex):
        m = {
            "x": np.ascontiguousarray(x[:, e]).reshape(T, Dx),
            "w1": np.ascontiguousarray(w1[e]),
            "b1": np.ascontiguousarray(b1[e]),
            "w2": np.ascontiguousarray(w2[e]),
        }
        if use_b2:
            m["b2"] = np.ascontiguousarray(b2[e])
        in_maps.append(m)

    br = run_bass_kernel_spmd(nc, in_maps, core_ids=list(range(Ex)),
                              trace=trace)
    ys = np.stack([br.results[e]["y"] for e in range(Ex)], axis=0)  # [E,T,D]
    out = ys.reshape(Ex, Bx, Nx, Dx).reshape(Bx, Ex, Nx, Dx)
    return (out, br) if trace else (out, None)


def kernel(x, w1, b1, w2, b2):
    out, _ = run_moe(np.asarray(x), np.asarray(w1), np.asarray(b1),
                     np.asarray(w2), np.asarray(b2))
    return out


# revision 8
# speedup vs baseline: 1.4546x; 1.0076x over previous
"""Expert-parallel MoE FFN kernel for Trainium2 (Bass/Tile).

Problem: y[b,e,n,:] = gelu(x[b,e,n,:] @ w1[e] + b1[e]) @ w2[e] + b2[e]
Shapes:  x (2,8,2048,1024), w1 (8,1024,4096), b1 (8,4096),
         w2 (8,4096,1024), b2 (8,1024)  -> out (2,8,2048,1024) fp32.

Sharding: expert-parallel, one expert per NeuronCore (8 cores).  Each core
processes its expert's 4096 tokens through the full FFN locally; no
cross-core communication.

Per-core dataflow (all matmuls in float32r at N=512 -> full PE rate):
  Phase 1:  xT = transpose(x_e)  (PE transpose, 128x128 tiles)
            hT[h,t] = gelu(sum_d w1[d,h] * xT[d,t] + b1[h])   (hT: [H,T])
            hT staged to DRAM (doesn't fit SBUF alongside weights).
  Phase 2:  y[t,d] = sum_h hT[h,t] * w2[h,d] (+ b2)
The hT (activation-transposed) layout means the big [H,T] intermediate is
produced and consumed with no transposes; only x needs a transpose on the
way in, and y comes out in natural [T,D] layout.

Scheduling notes:
 - GEMM1 runs group-outer (token-chunk groups) so the first-half xT tile can
   be freed at the GEMM1 midpoint; the leading slice of w2 loads into that
   space, hiding most of the phase-2 weight-load latency.
 - PSUM: GEMM1 uses [128, 2*512] psum tiles double-buffered (4 banks) so the
   GELU drain of one group overlaps the next group's matmuls; transposes use
   2 more banks; phase 2 reuses freed banks for [128, 1024] x2.
"""

import numpy as np
from contextlib import ExitStack

import concourse.bass as bass
import concourse.mybir as mybir
import concourse.tile as tile
from concourse import bacc
from concourse.bass_utils import run_bass_kernel_spmd
from concourse.masks import make_identity

P = 128
F32 = mybir.dt.float32
F32R = mybir.dt.float32r

# Full-size problem constants (hardcoded; the grading harness calls
# kernel(**inputs) with exactly these shapes).
B, E, N, D, H = 2, 8, 2048, 1024, 4096
N_CORES = 8


def emit_expert_ffn(tc, x, w1, b1, w2, b2, y, hT_st, T, D_, H_, TCH=512, DCH=512,
                    G=4, use_b2=False):
    """Emit one expert's FFN. x:[T,D] w1:[D,H] b1:[H] w2:[H,D] b2:[D] y:[T,D].

    hT_st: [H, T] DRAM staging tile (float32r) for the transposed activation.
    TCH: token chunk (matmul moving free dim) for GEMM1.
    DCH: output-dim chunk for GEMM2 (one PSUM bank).
    G:   token chunks per PSUM accumulation group in GEMM1.
    """
    nc = tc.nc
    NT = T // P          # token subtiles
    ND = D_ // P         # contraction tiles for GEMM1
    NH = H_ // P         # h tiles
    NCH = T // TCH       # token chunks
    NG = NCH // G        # chunk groups
    NDC = D_ // DCH      # output chunks for GEMM2
    NG_A = NG // 2       # groups reading the first-half xT tile
    W2A = min(16, NH)    # leading w2 h-tiles loaded during gemm1b

    assert T % P == 0 and D_ % P == 0 and H_ % P == 0
    assert T % TCH == 0 and NCH % G == 0 and D_ % DCH == 0 and NG % 2 == 0
    T_half = NG_A * G * TCH

    hT_r = hT_st.rearrange("(ht p) t -> p ht t", p=P)
    w2_r = w2.rearrange("(ht p) d -> p ht d", p=P)

    with ExitStack() as es:
        const_pool = es.enter_context(tc.tile_pool(name="const", bufs=1, side="right"))
        ident = const_pool.tile([P, P], F32)
        make_identity(nc, ident)
        b1_sb = const_pool.tile([P, NH], F32)
        nc.sync.dma_start(b1_sb[:], b1.rearrange("(ht p) -> p ht", p=P))
        if use_b2:
            b2_sb = const_pool.tile([P, D_], F32)
            nc.sync.dma_start(b2_sb[:], b2.unsqueeze(0).broadcast_to([P, D_]))

        # -------- Phase 1: xT transpose + hT = gelu(w1.T @ xT + b1) --------
        p1 = ExitStack()
        w1_pool = p1.enter_context(tc.tile_pool(name="w1_pool", bufs=2, side="left"))
        hout_pool = p1.enter_context(tc.tile_pool(name="hout_pool", bufs=2, side="left"))
        xTb_pool = p1.enter_context(tc.tile_pool(name="xTb_pool", bufs=1, side="left"))
        xTa_es = ExitStack()
        xTa_pool = xTa_es.enter_context(tc.tile_pool(name="xTa_pool", bufs=1, side="left"))

        xT_a = xTa_pool.tile([P, ND, T_half], F32R)
        xT_b = xTb_pool.tile([P, ND, T - T_half], F32R)

        def xT_slice(dt, t0, t1):
            if t1 <= T_half:
                return xT_a[:, dt, t0:t1]
            assert t0 >= T_half
            return xT_b[:, dt, t0 - T_half:t1 - T_half]

        with nc.named_scope("transpose"):
            with (
                tc.tile_pool(name="xraw_pool", bufs=3, side="left") as xraw_pool,
                tc.tile_pool(name="ptp_pool", bufs=4, space="PSUM",
                             side="left") as ptp_pool,
            ):
                for tsub in range(NT):
                    x_raw = xraw_pool.tile([P, D_], F32, name="x_raw")
                    DQ = max(P, D_ // 4)
                    for q0 in range(0, D_, DQ):
                        nc.sync.dma_start(
                            x_raw[:, q0:q0 + DQ],
                            x[tsub * P:(tsub + 1) * P, q0:q0 + DQ])
                    for dt in range(ND):
                        ptp = ptp_pool.tile([P, P], F32, name="ptp")
                        nc.tensor.transpose(ptp[:],
                                            x_raw[:, dt * P:(dt + 1) * P],
                                            ident[:])
                        nc.vector.tensor_copy(
                            xT_slice(dt, tsub * P, (tsub + 1) * P), ptp[:])

        # GEMM1 psum: [128, G*512] = 4 banks, double buffered = all 8 banks
        # (transpose psum pool closed above).
        ph_pool = p1.enter_context(tc.tile_pool(name="ph_pool", bufs=2,
                                                space="PSUM", side="left"))

        HTG = 4              # h-tiles per w1 chunk load (2KB DMA rows)
        assert NH % HTG == 0

        def gemm1_group(g, drip=None):
            for htg in range(NH // HTG):
                if drip:
                    for _ in range(2):
                        if drip:
                            drip.pop(0)()
                w1_t = w1_pool.tile([P, ND, HTG * P], F32R, name="w1_t")
                for dt in range(ND):
                    nc.sync.dma_start(
                        w1_t[:, dt, :],
                        w1[dt * P:(dt + 1) * P,
                           htg * HTG * P:(htg + 1) * HTG * P])
                for hl in range(HTG):
                    ht = htg * HTG + hl
                    psum_h = ph_pool.tile([P, G * TCH], F32, name="psum_h")
                    for dt in range(ND):
                        for i in range(G):
                            tc0 = (g * G + i) * TCH
                            nc.tensor.matmul(
                                psum_h[:, i * TCH:(i + 1) * TCH],
                                w1_t[:, dt, hl * P:(hl + 1) * P],
                                xT_slice(dt, tc0, tc0 + TCH),
                                start=(dt == 0), stop=(dt == ND - 1))
                    hT_out = hout_pool.tile([P, G * TCH], F32R, name="hT_out")
                    for i in range(G):
                        nc.scalar.activation(
                            hT_out[:, i * TCH:(i + 1) * TCH],
                            psum_h[:, i * TCH:(i + 1) * TCH],
                            mybir.ActivationFunctionType.Gelu_apprx_tanh,
                            bias=b1_sb[:, ht:ht + 1], scale=1.0)
                    nc.sync.dma_start(
                        hT_st[ht * P:(ht + 1) * P,
                              g * G * TCH:(g + 1) * G * TCH],
                        hT_out[:])

        with nc.named_scope("gemm1a"):
            for g in range(NG_A):
                gemm1_group(g)
        # first-half xT is dead; free its SBUF for the leading w2 slice.
        xTa_es.close()

        p2 = ExitStack()
        w2a_pool = p2.enter_context(tc.tile_pool(name="w2a_pool", bufs=1, side="right"))
        w2a = w2a_pool.tile([P, W2A, D_], F32R)

        def _w2a_load(ht):
            return lambda: nc.sync.dma_start(w2a[:, ht, :], w2_r[:, ht, :])

        drip = [_w2a_load(ht) for ht in range(W2A)]
        with nc.named_scope("gemm1b"):
            for g in range(NG_A, NG):
                gemm1_group(g, drip=drip)
        for thunk in drip:
            thunk()
        p1.close()

        # -------- Phase 2: y = hT.T @ w2 (+ b2) ----------------------------
        w2b_pool = p2.enter_context(tc.tile_pool(name="w2b_pool", bufs=1, side="right"))
        hTin_pool = p2.enter_context(tc.tile_pool(name="hTin_pool", bufs=2, side="right"))
        out_pool = p2.enter_context(tc.tile_pool(name="out_pool", bufs=3, side="right"))
        po_pool = p2.enter_context(tc.tile_pool(name="po_pool", bufs=2,
                                                space="PSUM", side="right"))
        if NH > W2A:
            w2b = w2b_pool.tile([P, NH - W2A, D_], F32R)
            for ht in range(W2A, NH):
                nc.sync.dma_start(w2b[:, ht - W2A, :], w2_r[:, ht, :])

        def w2_sb(ht):
            return w2a[:, ht, :] if ht < W2A else w2b[:, ht - W2A, :]

        with nc.named_scope("gemm2"):
            for tt in range(NT):
                hT_in = hTin_pool.tile([P, NH, P], F32R, name="hT_in")
                for hq in range(0, NH, NH // 4):
                    nc.sync.dma_start(
                        hT_in[:, hq:hq + NH // 4, :],
                        hT_r[:, hq:hq + NH // 4, tt * P:(tt + 1) * P])
                psum_o = po_pool.tile([P, D_], F32, name="psum_o")
                for ht in range(NH):
                    for dc in range(NDC):
                        nc.tensor.matmul(
                            psum_o[:, dc * DCH:(dc + 1) * DCH],
                            hT_in[:, ht, :],
                            w2_sb(ht)[:, dc * DCH:(dc + 1) * DCH],
                            start=(ht == 0), stop=(ht == NH - 1))
                out_sb = out_pool.tile([P, D_], F32, name="out_sb")
                if use_b2:
                    nc.vector.tensor_add(out_sb[:], psum_o[:], b2_sb[:])
                else:
                    nc.scalar.copy(out_sb[:], psum_o[:])
                nc.sync.dma_start(y[tt * P:(tt + 1) * P, :], out_sb[:])
        p2.close()


def build_module(T, D_, H_, TCH=512, DCH=512, use_b2=False):
    nc = bacc.Bacc(None, target_bir_lowering=False)
    x = nc.dram_tensor("x", [T, D_], F32, kind="ExternalInput")
    w1 = nc.dram_tensor("w1", [D_, H_], F32R, kind="ExternalInput")
    b1 = nc.dram_tensor("b1", [H_], F32, kind="ExternalInput")
    w2 = nc.dram_tensor("w2", [H_, D_], F32R, kind="ExternalInput")
    if use_b2:
        b2 = nc.dram_tensor("b2", [D_], F32, kind="ExternalInput")
    else:
        b2 = None
    y = nc.dram_tensor("y", [T, D_], F32, kind="ExternalOutput")

    with tile.TileContext(nc) as tc:
        with tc.tile_pool(name="dram_st", bufs=1, space="DRAM") as dram_pool:
            hT_st = dram_pool.tile([H_, T], F32R)
            emit_expert_ffn(tc, x[:], w1[:], b1[:], w2[:],
                            b2[:] if use_b2 else None, y[:], hT_st,
                            T, D_, H_, TCH=TCH, DCH=DCH, use_b2=use_b2)
    nc.compile()
    return nc


_module_cache = {}


def _get_module(key):
    if key not in _module_cache:
        T, D_, H_, use_b2 = key
        _module_cache[key] = build_module(T, D_, H_, use_b2=use_b2)
    return _module_cache[key]


def run_moe(x, w1, b1, w2, b2, trace=False):
    """x:(B,E,N,D) w1:(E,D,H) b1:(E,H) w2:(E,H,D) b2:(E,D) -> (B,E,N,D)."""
    Bx, Ex, Nx, Dx = x.shape
    Hx = w1.shape[2]
    T = Bx * Nx
    use_b2 = bool(np.any(b2))
    nc = _get_module((T, Dx, Hx, use_b2))

    in_maps = []
    for e in range(Ex):
        m = {
            "x": np.ascontiguousarray(x[:, e]).reshape(T, Dx),
            "w1": np.ascontiguousarray(w1[e]),
            "b1": np.ascontiguousarray(b1[e]),
            "w2": np.ascontiguousarray(w2[e]),
        }
        if use_b2:
            m["b2"] = np.ascontiguousarray(b2[e])
        in_maps.append(m)

    br = run_bass_kernel_spmd(nc, in_maps, core_ids=list(range(Ex)),
                              trace=trace)
    ys = np.stack([br.results[e]["y"] for e in range(Ex)], axis=0)  # [E,T,D]
    out = ys.reshape(Ex, Bx, Nx, Dx).reshape(Bx, Ex, Nx, Dx)
    return (out, br) if trace else (out, None)


def kernel(x, w1, b1, w2, b2):
    out, _ = run_moe(np.asarray(x), np.asarray(w1), np.asarray(b1),
                     np.asarray(w2), np.asarray(b2))
    return out


# revision 10
# speedup vs baseline: 1.4555x; 1.0006x over previous
"""Expert-parallel MoE FFN kernel for Trainium2 (Bass/Tile).

Problem: y[b,e,n,:] = gelu(x[b,e,n,:] @ w1[e] + b1[e]) @ w2[e] + b2[e]
Shapes:  x (2,8,2048,1024), w1 (8,1024,4096), b1 (8,4096),
         w2 (8,4096,1024), b2 (8,1024)  -> out (2,8,2048,1024) fp32.

Sharding: expert-parallel, one expert per NeuronCore (8 cores).  Each core
processes its expert's 4096 tokens through the full FFN locally; no
cross-core communication.

Per-core dataflow (all matmuls in float32r at N=512 -> full PE rate):
  Phase 1:  xT = transpose(x_e)  (PE transpose, 128x128 tiles)
            hT[h,t] = gelu(sum_d w1[d,h] * xT[d,t] + b1[h])   (hT: [H,T])
            hT staged to DRAM (doesn't fit SBUF alongside weights).
  Phase 2:  y[t,d] = sum_h hT[h,t] * w2[h,d] (+ b2)
The hT (activation-transposed) layout means the big [H,T] intermediate is
produced and consumed with no transposes; only x needs a transpose on the
way in, and y comes out in natural [T,D] layout.

Scheduling notes:
 - GEMM1 runs group-outer (token-chunk groups) so the first-half xT tile can
   be freed at the GEMM1 midpoint; the leading slice of w2 loads into that
   space, hiding most of the phase-2 weight-load latency.
 - PSUM: GEMM1 uses [128, 2*512] psum tiles double-buffered (4 banks) so the
   GELU drain of one group overlaps the next group's matmuls; transposes use
   2 more banks; phase 2 reuses freed banks for [128, 1024] x2.
"""

import numpy as np
from contextlib import ExitStack

import concourse.bass as bass
import concourse.mybir as mybir
import concourse.tile as tile
from concourse import bacc
from concourse.bass_utils import run_bass_kernel_spmd
from concourse.masks import make_identity

P = 128
F32 = mybir.dt.float32
F32R = mybir.dt.float32r

# Full-size problem constants (hardcoded; the grading harness calls
# kernel(**inputs) with exactly these shapes).
B, E, N, D, H = 2, 8, 2048, 1024, 4096
N_CORES = 8


def emit_expert_ffn(tc, x, w1, b1, w2, b2, y, hT_st, T, D_, H_, TCH=512, DCH=512,
                    G=4, use_b2=False):
    """Emit one expert's FFN. x:[T,D] w1:[D,H] b1:[H] w2:[H,D] b2:[D] y:[T,D].

    hT_st: [H, T] DRAM staging tile (float32r) for the transposed activation.
    TCH: token chunk (matmul moving free dim) for GEMM1.
    DCH: output-dim chunk for GEMM2 (one PSUM bank).
    G:   token chunks per PSUM accumulation group in GEMM1.
    """
    nc = tc.nc
    NT = T // P          # token subtiles
    ND = D_ // P         # contraction tiles for GEMM1
    NH = H_ // P         # h tiles
    NCH = T // TCH       # token chunks
    NG = NCH // G        # chunk groups
    NDC = D_ // DCH      # output chunks for GEMM2
    NG_A = NG // 2       # groups reading the first-half xT tile
    W2A = min(24, NH)    # leading w2 h-tiles loaded during gemm1b

    assert T % P == 0 and D_ % P == 0 and H_ % P == 0
    assert T % TCH == 0 and NCH % G == 0 and D_ % DCH == 0 and NG % 2 == 0
    T_half = NG_A * G * TCH

    hT_r = hT_st.rearrange("(ht p) t -> p ht t", p=P)
    w2_r = w2.rearrange("(ht p) d -> p ht d", p=P)

    with ExitStack() as es:
        const_pool = es.enter_context(tc.tile_pool(name="const", bufs=1, side="right"))
        ident = const_pool.tile([P, P], F32)
        make_identity(nc, ident)
        b1_sb = const_pool.tile([P, NH], F32)
        nc.sync.dma_start(b1_sb[:], b1.rearrange("(ht p) -> p ht", p=P))
        if use_b2:
            b2_sb = const_pool.tile([P, D_], F32)
            nc.sync.dma_start(b2_sb[:], b2.unsqueeze(0).broadcast_to([P, D_]))

        # -------- Phase 1: xT transpose + hT = gelu(w1.T @ xT + b1) --------
        p1 = ExitStack()
        w1_pool = p1.enter_context(tc.tile_pool(name="w1_pool", bufs=2, side="left"))
        hout_pool = p1.enter_context(tc.tile_pool(name="hout_pool", bufs=2, side="left"))
        xTb_pool = p1.enter_context(tc.tile_pool(name="xTb_pool", bufs=1, side="left"))
        xTa_es = ExitStack()
        xTa_pool = xTa_es.enter_context(tc.tile_pool(name="xTa_pool", bufs=1, side="left"))

        xT_a = xTa_pool.tile([P, ND, T_half], F32R)
        xT_b = xTb_pool.tile([P, ND, T - T_half], F32R)

        def xT_slice(dt, t0, t1):
            if t1 <= T_half:
                return xT_a[:, dt, t0:t1]
            assert t0 >= T_half
            return xT_b[:, dt, t0 - T_half:t1 - T_half]

        with nc.named_scope("transpose"):
            with (
                tc.tile_pool(name="xraw_pool", bufs=5, side="left") as xraw_pool,
                tc.tile_pool(name="ptp_pool", bufs=4, space="PSUM",
                             side="left") as ptp_pool,
            ):
                for tsub in range(NT):
                    x_raw = xraw_pool.tile([P, D_], F32, name="x_raw")
                    DQ = max(P, D_ // 4)
                    for q0 in range(0, D_, DQ):
                        nc.sync.dma_start(
                            x_raw[:, q0:q0 + DQ],
                            x[tsub * P:(tsub + 1) * P, q0:q0 + DQ])
                    for dt in range(ND):
                        ptp = ptp_pool.tile([P, P], F32, name="ptp")
                        nc.tensor.transpose(ptp[:],
                                            x_raw[:, dt * P:(dt + 1) * P],
                                            ident[:])
                        nc.vector.tensor_copy(
                            xT_slice(dt, tsub * P, (tsub + 1) * P), ptp[:])

        # GEMM1 psum: [128, G*512] = 4 banks, double buffered = all 8 banks
        # (transpose psum pool closed above).
        ph_pool = p1.enter_context(tc.tile_pool(name="ph_pool", bufs=2,
                                                space="PSUM", side="left"))

        HTG = 2              # h-tiles per w1 chunk load (1KB DMA rows)
        assert NH % HTG == 0

        def gemm1_group(g, drip=None):
            for htg in range(NH // HTG):
                if drip:
                    for _ in range(3):
                        if drip:
                            drip.pop(0)()
                w1_t = w1_pool.tile([P, ND, HTG * P], F32R, name="w1_t")
                for dt in range(ND):
                    nc.sync.dma_start(
                        w1_t[:, dt, :],
                        w1[dt * P:(dt + 1) * P,
                           htg * HTG * P:(htg + 1) * HTG * P])
                for hl in range(HTG):
                    ht = htg * HTG + hl
                    psum_h = ph_pool.tile([P, G * TCH], F32, name="psum_h")
                    for dt in range(ND):
                        for i in range(G):
                            tc0 = (g * G + i) * TCH
                            nc.tensor.matmul(
                                psum_h[:, i * TCH:(i + 1) * TCH],
                                w1_t[:, dt, hl * P:(hl + 1) * P],
                                xT_slice(dt, tc0, tc0 + TCH),
                                start=(dt == 0), stop=(dt == ND - 1))
                    hT_out = hout_pool.tile([P, G * TCH], F32R, name="hT_out")
                    for i in range(G):
                        nc.scalar.activation(
                            hT_out[:, i * TCH:(i + 1) * TCH],
                            psum_h[:, i * TCH:(i + 1) * TCH],
                            mybir.ActivationFunctionType.Gelu_apprx_tanh,
                            bias=b1_sb[:, ht:ht + 1], scale=1.0)
                    nc.sync.dma_start(
                        hT_st[ht * P:(ht + 1) * P,
                              g * G * TCH:(g + 1) * G * TCH],
                        hT_out[:])

        with nc.named_scope("gemm1a"):
            for g in range(NG_A):
                gemm1_group(g)
        # first-half xT is dead; free its SBUF for the leading w2 slice.
        xTa_es.close()

        p2 = ExitStack()
        w2a_pool = p2.enter_context(tc.tile_pool(name="w2a_pool", bufs=1, side="right"))
        w2a = w2a_pool.tile([P, W2A, D_], F32R)

        def _w2a_load(ht):
            return lambda: nc.sync.dma_start(w2a[:, ht, :], w2_r[:, ht, :])

        drip = [_w2a_load(ht) for ht in range(W2A)]
        with nc.named_scope("gemm1b"):
            for g in range(NG_A, NG):
                gemm1_group(g, drip=drip)
        for thunk in drip:
            thunk()
        p1.close()

        # -------- Phase 2: y = hT.T @ w2 (+ b2) ----------------------------
        w2b_pool = p2.enter_context(tc.tile_pool(name="w2b_pool", bufs=1, side="right"))
        hTin_pool = p2.enter_context(tc.tile_pool(name="hTin_pool", bufs=3, side="right"))
        out_pool = p2.enter_context(tc.tile_pool(name="out_pool", bufs=3, side="right"))
        po_pool = p2.enter_context(tc.tile_pool(name="po_pool", bufs=2,
                                                space="PSUM", side="right"))
        w2b = (w2b_pool.tile([P, NH - W2A, D_], F32R, name="w2b")
               if NH > W2A else None)

        def w2_sb(ht):
            return w2a[:, ht, :] if ht < W2A else w2b[:, ht - W2A, :]

        with nc.named_scope("gemm2"):
            for tt in range(NT):
                hT_in = hTin_pool.tile([P, NH, P], F32R, name="hT_in")
                for hq in range(0, NH, NH // 4):
                    nc.sync.dma_start(
                        hT_in[:, hq:hq + NH // 4, :],
                        hT_r[:, hq:hq + NH // 4, tt * P:(tt + 1) * P])
                if tt == 0 and w2b is not None:
                    # emitted after the first hT load so those DMAs win the
                    # queue race; needed from the ht=W2A matmul onward.
                    for ht in range(W2A, NH):
                        nc.sync.dma_start(w2b[:, ht - W2A, :], w2_r[:, ht, :])
                psum_o = po_pool.tile([P, D_], F32, name="psum_o")
                for ht in range(NH):
                    for dc in range(NDC):
                        nc.tensor.matmul(
                            psum_o[:, dc * DCH:(dc + 1) * DCH],
                            hT_in[:, ht, :],
                            w2_sb(ht)[:, dc * DCH:(dc + 1) * DCH],
                            start=(ht == 0), stop=(ht == NH - 1))
                out_sb = out_pool.tile([P, D_], F32, name="out_sb")
                if use_b2:
                    nc.vector.tensor_add(out_sb[:], psum_o[:], b2_sb[:])
                else:
                    nc.scalar.copy(out_sb[:], psum_o[:])
                nc.sync.dma_start(y[tt * P:(tt + 1) * P, :], out_sb[:])
        p2.close()


def build_module(T, D_, H_, TCH=512, DCH=512, use_b2=False):
    nc = bacc.Bacc(None, target_bir_lowering=False)
    x = nc.dram_tensor("x", [T, D_], F32, kind="ExternalInput")
    w1 = nc.dram_tensor("w1", [D_, H_], F32R, kind="ExternalInput")
    b1 = nc.dram_tensor("b1", [H_], F32, kind="ExternalInput")
    w2 = nc.dram_tensor("w2", [H_, D_], F32R, kind="ExternalInput")
    if use_b2:
        b2 = nc.dram_tensor("b2", [D_], F32, kind="ExternalInput")
    else:
        b2 = None
    y = nc.dram_tensor("y", [T, D_], F32, kind="ExternalOutput")

    with tile.TileContext(nc) as tc:
        with tc.tile_pool(name="dram_st", bufs=1, space="DRAM") as dram_pool:
            hT_st = dram_pool.tile([H_, T], F32R)
            emit_expert_ffn(tc, x[:], w1[:], b1[:], w2[:],
                            b2[:] if use_b2 else None, y[:], hT_st,
                            T, D_, H_, TCH=TCH, DCH=DCH, use_b2=use_b2)
    nc.compile()
    return nc


_module_cache = {}


def _get_module(key):
    if key not in _module_cache:
        T, D_, H_, use_b2 = key
        _module_cache[key] = build_module(T, D_, H_, use_b2=use_b2)
    return _module_cache[key]


def run_moe(x, w1, b1, w2, b2, trace=False):
    """x:(B,E,N,D) w1:(E,D,H) b1:(E,H) w2:(E,H,D) b2:(E,D) -> (B,E,N,D)."""
    Bx, Ex, Nx, Dx = x.shape
    Hx = w1.shape[2]
    T = Bx * Nx
    use_b2 = bool(np.any(b2))
    nc = _get_module((T, Dx, Hx, use_b2))

    in_maps = []
    for e in range(Ex):
        m = {
            "x": np.ascontiguousarray(x[:, e]).reshape(T, Dx),
            "w1": np.ascontiguousarray(w1[e]),
            "b1": np.ascontiguousarray(b1[e]),
            "w2": np.ascontiguousarray(w2[e]),
        }
        if use_b2:
            m["b2"] = np.ascontiguousarray(b2[e])
        in_maps.append(m)

    br = run_bass_kernel_spmd(nc, in_maps, core_ids=list(range(Ex)),
                              trace=trace)
    ys = np.stack([br.results[e]["y"] for e in range(Ex)], axis=0)  # [E,T,D]
    out = ys.reshape(Ex, Bx, Nx, Dx).reshape(Bx, Ex, Nx, Dx)
    return (out, br) if trace else (out, None)


def kernel(x, w1, b1, w2, b2):
    out, _ = run_moe(np.asarray(x), np.asarray(w1), np.asarray(b1),
                     np.asarray(w2), np.asarray(b2))
    return out


# revision 12
# speedup vs baseline: 1.4622x; 1.0046x over previous
"""Expert-parallel MoE FFN kernel for Trainium2 (Bass/Tile).

Problem: y[b,e,n,:] = gelu(x[b,e,n,:] @ w1[e] + b1[e]) @ w2[e] + b2[e]
Shapes:  x (2,8,2048,1024), w1 (8,1024,4096), b1 (8,4096),
         w2 (8,4096,1024), b2 (8,1024)  -> out (2,8,2048,1024) fp32.

Sharding: expert-parallel, one expert per NeuronCore (8 cores).  Each core
processes its expert's 4096 tokens through the full FFN locally; no
cross-core communication.

Per-core dataflow (all matmuls in float32r at N=512 -> full PE rate):
  Phase 1:  xT = transpose(x_e)  (PE transpose, 128x128 tiles)
            hT[h,t] = gelu(sum_d w1[d,h] * xT[d,t] + b1[h])   (hT: [H,T])
            hT staged to DRAM (doesn't fit SBUF alongside weights).
  Phase 2:  y[t,d] = sum_h hT[h,t] * w2[h,d] (+ b2)
The hT (activation-transposed) layout means the big [H,T] intermediate is
produced and consumed with no transposes; only x needs a transpose on the
way in, and y comes out in natural [T,D] layout.

Scheduling notes:
 - GEMM1 runs group-outer (token-chunk groups) so the first-half xT tile can
   be freed at the GEMM1 midpoint; the leading slice of w2 loads into that
   space, hiding most of the phase-2 weight-load latency.
 - PSUM: GEMM1 uses [128, 2*512] psum tiles double-buffered (4 banks) so the
   GELU drain of one group overlaps the next group's matmuls; transposes use
   2 more banks; phase 2 reuses freed banks for [128, 1024] x2.
"""

import numpy as np
from contextlib import ExitStack

import concourse.bass as bass
import concourse.mybir as mybir
import concourse.tile as tile
from concourse import bacc
from concourse.bass_utils import run_bass_kernel_spmd
from concourse.masks import make_identity

P = 128
F32 = mybir.dt.float32
F32R = mybir.dt.float32r

# Full-size problem constants (hardcoded; the grading harness calls
# kernel(**inputs) with exactly these shapes).
B, E, N, D, H = 2, 8, 2048, 1024, 4096
N_CORES = 8


def emit_expert_ffn(tc, x, w1, b1, w2, b2, y, hT_st, T, D_, H_, TCH=512, DCH=512,
                    G=4, use_b2=False):
    """Emit one expert's FFN. x:[T,D] w1:[D,H] b1:[H] w2:[H,D] b2:[D] y:[T,D].

    hT_st: [H, T] DRAM staging tile (float32r) for the transposed activation.
    TCH: token chunk (matmul moving free dim) for GEMM1.
    DCH: output-dim chunk for GEMM2 (one PSUM bank).
    G:   token chunks per PSUM accumulation group in GEMM1.
    """
    nc = tc.nc
    NT = T // P          # token subtiles
    ND = D_ // P         # contraction tiles for GEMM1
    NH = H_ // P         # h tiles
    NCH = T // TCH       # token chunks
    NG = NCH // G        # chunk groups
    NDC = D_ // DCH      # output chunks for GEMM2
    NG_A = NG // 2       # groups reading the first-half xT tile
    W2A = min(24, NH)    # leading w2 h-tiles loaded during gemm1b

    assert T % P == 0 and D_ % P == 0 and H_ % P == 0
    assert T % TCH == 0 and NCH % G == 0 and D_ % DCH == 0 and NG % 2 == 0
    T_half = NG_A * G * TCH

    hT_r = hT_st.rearrange("(ht p) t -> p ht t", p=P)
    w2_r = w2.rearrange("(ht p) d -> p ht d", p=P)

    with ExitStack() as es:
        const_pool = es.enter_context(tc.tile_pool(name="const", bufs=1, side="right"))
        ident = const_pool.tile([P, P], F32)
        make_identity(nc, ident)
        b1_sb = const_pool.tile([P, NH], F32)
        nc.sync.dma_start(b1_sb[:], b1.rearrange("(ht p) -> p ht", p=P))
        if use_b2:
            b2_sb = const_pool.tile([P, D_], F32)
            nc.sync.dma_start(b2_sb[:], b2.unsqueeze(0).broadcast_to([P, D_]))

        # -------- Phase 1: xT transpose + hT = gelu(w1.T @ xT + b1) --------
        p1 = ExitStack()
        w1_pool = p1.enter_context(tc.tile_pool(name="w1_pool", bufs=2, side="left"))
        hout_pool = p1.enter_context(tc.tile_pool(name="hout_pool", bufs=2, side="left"))
        xTb_pool = p1.enter_context(tc.tile_pool(name="xTb_pool", bufs=1, side="left"))
        xTa_es = ExitStack()
        xTa_pool = xTa_es.enter_context(tc.tile_pool(name="xTa_pool", bufs=1, side="left"))

        xT_a = xTa_pool.tile([P, ND, T_half], F32R)
        xT_b = xTb_pool.tile([P, ND, T - T_half], F32R)

        def xT_slice(dt, t0, t1):
            if t1 <= T_half:
                return xT_a[:, dt, t0:t1]
            assert t0 >= T_half
            return xT_b[:, dt, t0 - T_half:t1 - T_half]

        with nc.named_scope("transpose"):
            with (
                tc.tile_pool(name="xraw_pool", bufs=5, side="left") as xraw_pool,
                tc.tile_pool(name="ptp_pool", bufs=6, space="PSUM",
                             side="left") as ptp_pool,
            ):
                for tsub in range(NT):
                    x_raw = xraw_pool.tile([P, D_], F32, name="x_raw")
                    DQ = max(P, D_ // 4)
                    for q0 in range(0, D_, DQ):
                        nc.sync.dma_start(
                            x_raw[:, q0:q0 + DQ],
                            x[tsub * P:(tsub + 1) * P, q0:q0 + DQ])
                    for dt in range(ND):
                        ptp = ptp_pool.tile([P, P], F32, name="ptp")
                        nc.tensor.transpose(ptp[:],
                                            x_raw[:, dt * P:(dt + 1) * P],
                                            ident[:])
                        nc.vector.tensor_copy(
                            xT_slice(dt, tsub * P, (tsub + 1) * P), ptp[:])

        # GEMM1 psum: [128, G*512] = 4 banks, double buffered = all 8 banks
        # (transpose psum pool closed above).
        ph_pool = p1.enter_context(tc.tile_pool(name="ph_pool", bufs=2,
                                                space="PSUM", side="left"))

        HTG = 2              # h-tiles per w1 chunk load (1KB DMA rows)
        assert NH % HTG == 0

        def gemm1_group(g, drip=None):
            for htg in range(NH // HTG):
                if drip:
                    for _ in range(3):
                        if drip:
                            drip.pop(0)()
                w1_t = w1_pool.tile([P, ND, HTG * P], F32R, name="w1_t")
                for dt in range(ND):
                    nc.sync.dma_start(
                        w1_t[:, dt, :],
                        w1[dt * P:(dt + 1) * P,
                           htg * HTG * P:(htg + 1) * HTG * P])
                for hl in range(HTG):
                    ht = htg * HTG + hl
                    psum_h = ph_pool.tile([P, G * TCH], F32, name="psum_h")
                    for dt in range(ND):
                        for i in range(G):
                            tc0 = (g * G + i) * TCH
                            nc.tensor.matmul(
                                psum_h[:, i * TCH:(i + 1) * TCH],
                                w1_t[:, dt, hl * P:(hl + 1) * P],
                                xT_slice(dt, tc0, tc0 + TCH),
                                start=(dt == 0), stop=(dt == ND - 1))
                    hT_out = hout_pool.tile([P, G * TCH], F32R, name="hT_out")
                    for i in range(G):
                        nc.scalar.activation(
                            hT_out[:, i * TCH:(i + 1) * TCH],
                            psum_h[:, i * TCH:(i + 1) * TCH],
                            mybir.ActivationFunctionType.Gelu_apprx_tanh,
                            bias=b1_sb[:, ht:ht + 1], scale=1.0)
                    nc.sync.dma_start(
                        hT_st[ht * P:(ht + 1) * P,
                              g * G * TCH:(g + 1) * G * TCH],
                        hT_out[:])

        with nc.named_scope("gemm1a"):
            for g in range(NG_A):
                gemm1_group(g)
        # first-half xT is dead; free its SBUF for the leading w2 slice.
        xTa_es.close()

        p2 = ExitStack()
        w2a_pool = p2.enter_context(tc.tile_pool(name="w2a_pool", bufs=1, side="right"))
        w2a = w2a_pool.tile([P, W2A, D_], F32R)

        def _w2a_load(ht):
            return lambda: nc.sync.dma_start(w2a[:, ht, :], w2_r[:, ht, :])

        drip = [_w2a_load(ht) for ht in range(W2A)]
        with nc.named_scope("gemm1b"):
            for g in range(NG_A, NG):
                gemm1_group(g, drip=drip)
        for thunk in drip:
            thunk()
        p1.close()

        # -------- Phase 2: y = hT.T @ w2 (+ b2) ----------------------------
        w2b_pool = p2.enter_context(tc.tile_pool(name="w2b_pool", bufs=1, side="right"))
        hTin_pool = p2.enter_context(tc.tile_pool(name="hTin_pool", bufs=3, side="right"))
        out_pool = p2.enter_context(tc.tile_pool(name="out_pool", bufs=3, side="right"))
        po_pool = p2.enter_context(tc.tile_pool(name="po_pool", bufs=2,
                                                space="PSUM", side="right"))
        w2b = (w2b_pool.tile([P, NH - W2A, D_], F32R, name="w2b")
               if NH > W2A else None)

        def w2_sb(ht):
            return w2a[:, ht, :] if ht < W2A else w2b[:, ht - W2A, :]

        with nc.named_scope("gemm2"):
            for tt in range(NT):
                hT_in = hTin_pool.tile([P, NH, P], F32R, name="hT_in")
                for hq in range(0, NH, NH // 4):
                    nc.sync.dma_start(
                        hT_in[:, hq:hq + NH // 4, :],
                        hT_r[:, hq:hq + NH // 4, tt * P:(tt + 1) * P])
                if tt == 0 and w2b is not None:
                    # emitted after the first hT load so those DMAs win the
                    # queue race; needed from the ht=W2A matmul onward.
                    for ht in range(W2A, NH):
                        nc.sync.dma_start(w2b[:, ht - W2A, :], w2_r[:, ht, :])
                psum_o = po_pool.tile([P, D_], F32, name="psum_o")
                for ht in range(NH):
                    for dc in range(NDC):
                        nc.tensor.matmul(
                            psum_o[:, dc * DCH:(dc + 1) * DCH],
                            hT_in[:, ht, :],
                            w2_sb(ht)[:, dc * DCH:(dc + 1) * DCH],
                            start=(ht == 0), stop=(ht == NH - 1))
                out_sb = out_pool.tile([P, D_], F32, name="out_sb")
                for dc in range(NDC):
                    sl = slice(dc * DCH, (dc + 1) * DCH)
                    if use_b2:
                        nc.vector.tensor_add(out_sb[:, sl], psum_o[:, sl],
                                             b2_sb[:, sl])
                    else:
                        nc.scalar.copy(out_sb[:, sl], psum_o[:, sl])
                    nc.sync.dma_start(y[tt * P:(tt + 1) * P, sl],
                                      out_sb[:, sl])
        p2.close()


def build_module(T, D_, H_, TCH=512, DCH=512, use_b2=False):
    nc = bacc.Bacc(None, target_bir_lowering=False)
    x = nc.dram_tensor("x", [T, D_], F32, kind="ExternalInput")
    w1 = nc.dram_tensor("w1", [D_, H_], F32R, kind="ExternalInput")
    b1 = nc.dram_tensor("b1", [H_], F32, kind="ExternalInput")
    w2 = nc.dram_tensor("w2", [H_, D_], F32R, kind="ExternalInput")
    if use_b2:
        b2 = nc.dram_tensor("b2", [D_], F32, kind="ExternalInput")
    else:
        b2 = None
    y = nc.dram_tensor("y", [T, D_], F32, kind="ExternalOutput")

    with tile.TileContext(nc) as tc:
        with tc.tile_pool(name="dram_st", bufs=1, space="DRAM") as dram_pool:
            hT_st = dram_pool.tile([H_, T], F32R)
            emit_expert_ffn(tc, x[:], w1[:], b1[:], w2[:],
                            b2[:] if use_b2 else None, y[:], hT_st,
                            T, D_, H_, TCH=TCH, DCH=DCH, use_b2=use_b2)
    nc.compile()
    return nc


_module_cache = {}


def _get_module(key):
    if key not in _module_cache:
        T, D_, H_, use_b2 = key
        _module_cache[key] = build_module(T, D_, H_, use_b2=use_b2)
    return _module_cache[key]


def run_moe(x, w1, b1, w2, b2, trace=False):
    """x:(B,E,N,D) w1:(E,D,H) b1:(E,H) w2:(E,H,D) b2:(E,D) -> (B,E,N,D)."""
    Bx, Ex, Nx, Dx = x.shape
    Hx = w1.shape[2]
    T = Bx * Nx
    use_b2 = bool(np.any(b2))
    nc = _get_module((T, Dx, Hx, use_b2))

    in_maps = []
    for e in range(Ex):
        m = {
            "x": np.ascontiguousarray(x[:, e]).reshape(T, Dx),
            "w1": np.ascontiguousarray(w1[e]),
            "b1": np.ascontiguousarray(b1[e]),
            "w2": np.ascontiguousarray(w2[e]),
        }
        if use_b2:
            m["b2"] = np.ascontiguousarray(b2[e])
        in_maps.append(m)

    br = run_bass_kernel_spmd(nc, in_maps, core_ids=list(range(Ex)),
                              trace=trace)
    ys = np.stack([br.results[e]["y"] for e in range(Ex)], axis=0)  # [E,T,D]
    out = ys.reshape(Ex, Bx, Nx, Dx).reshape(Bx, Ex, Nx, Dx)
    return (out, br) if trace else (out, None)


def kernel(x, w1, b1, w2, b2):
    out, _ = run_moe(np.asarray(x), np.asarray(w1), np.asarray(b1),
                     np.asarray(w2), np.asarray(b2))
    return out
